# revision 3
# baseline (speedup 1.0000x reference)
"""GroupPretrainHead on 8 NeuronCores (Trainium2, Bass/Tile).

Expert-parallel sharding: core g owns group g's decoder (W[g], b[g]) and
processes exactly the samples routed to group g. The host does the routing
permutation (gather/scatter of rows = the MoE dispatch/combine step); the
device does all FLOPs: out.T = W[g] @ h.T + b[g] as a K-accumulated matmul.

v2 vs baseline:
  - h and W shipped as bf16 (PSUM accumulation stays f32) -> half the HBM
    traffic; rel err ~3e-3 vs the 2e-2 gate.
  - C = max group count rounded to 16 (not 128) -> ~10% fewer bytes.
  - h is chunk-blocked on the host into NCHUNK column-chunks, each a single
    fully-contiguous [128, KT*s] DMA; matmuls for a chunk need only that
    chunk, so compute+store pipeline behind the DMA stream.
  - Each chunk owns its own PSUM bank (<=8 chunks); no PSUM reuse means the
    first matmul of a chunk waits ONLY on its DMA (the one-sem-wait ISA
    limit). The w-DMA wait is absorbed by a warmup ldweights, the bias-DMA
    wait by a warmup DVE copy.
  - Small first chunk (compute starts ~1.5us earlier) and small last chunk
    (short matmul+store tail after the final DMA lands).
  - Stores + weight loads on the ACT HWDGE ring, h stream on the SP ring.

Device-side layout per core:
  hP   [128, KT*C] bf16 -- chunk-blocked: cols [KT*o, KT*(o+s)) hold chunk
                           (o,s) as KT k-tile segments of s samples each
  wT   [128, KT*64] bf16 -- W[g] transposed to [d-partition, (ktile j)]
  bias [64, 1]      f32
  outT [64, C]      f32  -- preds.T for this group's samples
"""

import numpy as np

N_GROUPS = 8
D_MODEL = 2048
MAX_GS = 64
PART = 128
KT = D_MODEL // PART  # 16

TRACE = False
LAST_EXEC_NS = None
LAST_RESULTS = None

_nc_cache = {}


def _chunk_sizes(C):
    """<=8 column-chunks summing to C: small head (early compute start),
    small tail (short post-DMA epilogue), fat middle. All multiples of 16;
    each must fit one PSUM bank (<=512 f32)."""
    if C <= 128:
        return [C]
    head = tail = 32
    mid_total = C - head - tail
    n_mid = 6
    base = (mid_total // n_mid) // 16 * 16
    sizes_mid = [base] * n_mid
    rem = mid_total - base * n_mid
    i = 0
    while rem > 0:
        add = min(16, rem)
        sizes_mid[i % n_mid] += add
        rem -= add
        i += 1
    sizes = [head] + sizes_mid + [tail]
    assert sum(sizes) == C and all(s <= 512 for s in sizes)
    return sizes


def _make_tile_context_cls():
    import concourse.mybir as mybir
    from concourse.tile import TileContext
    from concourse.vector_clock import ScopedClock

    class SplitDrainTileContext(TileContext):
        """This container's walrus encodes at most ONE semaphore wait per
        instruction; Tile's kernel-tail drain aggregates every outstanding
        sem onto a single InstDrain, which fails codegen. Split it into a
        chain of one-wait drains."""

        def _drain_and_barrier(self, tick_clock, wait_clock):
            drain_inst = self.nc.sync.drain()
            wait_clock.add_sem_waits(
                drain_inst.ins, ScopedClock({None: tick_clock.global_clock})
            )
            si = drain_inst.ins.sync_info
            waits = list(si.on_wait) if si else []
            if len(waits) > 1:
                si.on_wait = waits[:1]
                drain_inst.ins.sync_info = si
                for w in waits[1:]:
                    d2 = self.nc.sync.drain()
                    d2.ins.sync_info = mybir.SyncInfo(on_wait=[w], on_update=[])
            self.nc.all_engine_barrier()
            popped = self.nc._tile_sem_poison_stack.pop()
            assert popped is self._sem_poison
            self.nc.clear_and_free_semaphores(list(self.sems.allocated().values()))
            self.nc.all_engine_barrier()

    return SplitDrainTileContext


def _build_nc(C, sizes):
    import concourse.bass as bass
    import concourse.mybir as mybir

    TileContext = _make_tile_context_cls()

    f32 = mybir.dt.float32
    bf16 = mybir.dt.bfloat16
    nc = bass.Bass()

    hP = nc.declare_dram_parameter("hP", [PART, KT * C], bf16, isOutput=False)
    wT = nc.declare_dram_parameter("wT", [PART, KT * MAX_GS], bf16, isOutput=False)
    bias = nc.declare_dram_parameter("bias", [MAX_GS, 1], f32, isOutput=False)
    outT = nc.declare_dram_parameter("outT", [MAX_GS, C], f32, isOutput=True)

    offs = [0]
    for s in sizes[:-1]:
        offs.append(offs[-1] + s)

    with TileContext(nc) as tc:
        with (
            tc.tile_pool(name="const", bufs=1) as constp,
            tc.tile_pool(name="h", bufs=1) as hp,
            tc.tile_pool(name="psum", bufs=1, space=bass.MemorySpace.PSUM) as pp,
            tc.tile_pool(name="out", bufs=1) as op,
        ):
            w_sb = constp.tile([PART, KT * MAX_GS], bf16, tag="w")
            nc.scalar.dma_start(w_sb[:], wT[:])
            b_sb = constp.tile([MAX_GS, 1], f32, tag="b")
            nc.scalar.dma_start(b_sb[:], bias[:])

            # Absorb the w/b DMA completion waits on their consuming engines
            # so every real matmul/add carries exactly one semaphore wait
            # (the walrus encodes at most one per instruction).
            nc.tensor.ldweights(w_sb[:, 0:MAX_GS])
            b_warm = constp.tile([MAX_GS, 1], f32, tag="bwarm")
            nc.vector.tensor_copy(b_warm[:], b_sb[:])

            o_sb = op.tile([MAX_GS, C], f32, tag="o")

            for i, (o, s) in enumerate(zip(offs, sizes)):
                h_sb = hp.tile([PART, KT * s], bf16, tag=f"h{i}")
                nc.sync.dma_start(h_sb[:], hP[:, KT * o : KT * (o + s)])
                ps = pp.tile([MAX_GS, s], f32, tag=f"ps{i}", name=f"ps{i}")
                for t in range(KT):
                    nc.tensor.matmul(
                        ps[:, :],
                        w_sb[:, t * MAX_GS : (t + 1) * MAX_GS],
                        h_sb[:, t * s : (t + 1) * s],
                        start=(t == 0),
                        stop=(t == KT - 1),
                    )
                nc.vector.tensor_scalar_add(o_sb[:, o : o + s], ps[:, :], b_sb[:])
                # SWDGE: stores use the DMASW completion lanes, so they never
                # collide with the h-stream's DMAHW lane reuse (each HWDGE
                # lane reuse adds a second sem wait, which fails codegen).
                nc.gpsimd.dma_start(outT[:, o : o + s], o_sb[:, o : o + s])

    return nc


def kernel(**inputs):
    global LAST_EXEC_NS, LAST_RESULTS
    import ml_dtypes
    from concourse.bass_utils import run_bass_kernel_spmd

    bf16 = ml_dtypes.bfloat16

    hidden = np.asarray(inputs["hidden"], dtype=np.float32)
    idx = np.asarray(inputs["chosen_group_idx"]).astype(np.int64)
    W = np.asarray(inputs["W"], dtype=np.float32)
    b = np.asarray(inputs["b"], dtype=np.float32)
    gs = np.asarray(inputs["group_sizes"])

    B = hidden.shape[0]
    counts = np.bincount(idx, minlength=N_GROUPS)
    C = max(16, int(-(-counts.max() // 16)) * 16)
    sizes = _chunk_sizes(C)
    offs = np.concatenate([[0], np.cumsum(sizes)[:-1]]).astype(int)

    positions = [np.nonzero(idx == g)[0] for g in range(N_GROUPS)]

    in_maps = []
    for g in range(N_GROUPS):
        pos = positions[g]
        hg = np.zeros((C, D_MODEL), bf16)
        hg[: len(pos)] = hidden[pos, g, :].astype(bf16)
        blocks = [
            np.ascontiguousarray(
                hg[o : o + s].reshape(s, KT, PART).transpose(2, 1, 0)
            ).reshape(PART, KT * s)
            for o, s in zip(offs, sizes)
        ]
        hPm = np.ascontiguousarray(np.concatenate(blocks, axis=1))
        wTm = np.ascontiguousarray(
            W[g].astype(bf16).reshape(MAX_GS, KT, PART).transpose(2, 1, 0)
        ).reshape(PART, KT * MAX_GS)
        biasm = np.ascontiguousarray(b[g][:, None])
        in_maps.append({"hP": hPm, "wT": wTm, "bias": biasm})

    key = (C, tuple(sizes))
    if key not in _nc_cache:
        _nc_cache[key] = _build_nc(C, sizes)
    nc = _nc_cache[key]

    res = run_bass_kernel_spmd(nc, in_maps, list(range(N_GROUPS)), trace=TRACE)
    LAST_EXEC_NS = res.exec_time_ns
    LAST_RESULTS = res

    preds = np.zeros((B, MAX_GS), np.float32)
    for g in range(N_GROUPS):
        pos = positions[g]
        outT = res.results[g]["outT"]  # [64, C]
        preds[pos] = outT.T[: len(pos)]

    valid = np.arange(MAX_GS)[None, :] < gs[idx][:, None]
    preds = np.where(valid, preds, np.float32(0.0))
    return preds, valid
